# revision 11
# baseline (speedup 1.0000x reference)
# Trainium2 Bass kernel for nn_CAM: channel-attention module
#   x: (16, 512, 64, 64) f32, Wc: (512, 512) f32
#   q = Wc @ x_flat; E = q @ q^T; att = softmax(E, -1); out = att @ x_flat
#
# Sharding: data-parallel over batch B across 8 cores (2 batches/core),
# Wc replicated. Per batch, on-chip:
#   qT[n,o] = sum_c x[c,n] WcT[c,o]            (bf16 matmul)
#   E[c,d]  = sum_n qT[n,c] qT[n,d]            (bf16 matmul, fp32 PSUM)
#   P       = exp(E - rowmax(E)), s = rowsum   (ACT, direct from PSUM)
#   A'      = P - diag(s)                      (exact when softmax==I)
#   out     = diag(1/s) A'^T.T @ bf16(x) + x   (bf16 matmul + fused DVE)
# This factorization of out = softmax(E) @ x keeps the value path exact:
# for this problem softmax(E) is numerically the identity in fp32
# (diag(E) ~ 4096, off-diag gap > 2000, exp underflows), so A' == 0 and
# out == x bitwise; any deviation is still tracked faithfully through
# the correction matmul.

from contextlib import ExitStack

import numpy as np
import ml_dtypes

import concourse.bass as bass
import concourse.bacc as bacc
import concourse.mybir as mybir
import concourse.tile as tile
from concourse.bass_utils import run_bass_kernel_spmd
from concourse.masks import make_identity

N_CORES = 8
B, C, HW = 16, 512, 4096
H = W = 64
BPC = B // N_CORES  # batches per core
P = 128
CB = C // P         # 4 channel blocks
NK = HW // P        # 32 n-blocks (K steps for E)
NJ = HW // 512      # 8 n-chunks of 512
F32 = mybir.dt.float32
BF16 = mybir.dt.bfloat16
AX = mybir.AxisListType.X
EXP = mybir.ActivationFunctionType.Exp
MUL = mybir.AluOpType.mult
ADD = mybir.AluOpType.add


def _batch_body(ctx, tc, pools, xv, ov, wct_sb, ident_bf):
    """Emit one batch's pipeline. xv/ov are [P, CB, HW] DRAM views."""
    nc = tc.nc
    (xb_pool, qt_pool, ab_pool, at_pool, si_pool,
     stat_pool, xf2_pool, out_pool, qtps, epsum, atps, ops) = pools

    # ---- Phase A: load x fp32 once (HWDGE), cast to bf16 on-chip ----
    # x fp32 chunks stay resident: consumed by the bf16 cast now and by
    # the phase-E final add later, so HBM reads x only once.
    xb = xb_pool.tile([P, CB, HW], BF16, tag="xb")
    xf2 = []
    for j in range(NJ):
        t = xf2_pool.tile([P, CB, 512], F32, tag="xf2", name=f"xf2_{j}")
        nc.sync.dma_start(t[:], xv[:, :, bass.ts(j, 512)])
        xf2.append(t)
        for cb in range(CB):
            nc.gpsimd.tensor_copy(out=xb[:, cb, bass.ts(j, 512)],
                                  in_=t[:, cb, :])

    # ---- Phase B: qT and E, interleaved over 32 n-blocks ----
    e_ps = [epsum.tile([P, 512], F32, tag=f"E{ci}", name=f"E{ci}")
            for ci in range(CB)]
    for k in range(NK):
        qt_ps = qtps.tile([P, 512], F32, tag="qtps")
        for cb in range(CB):
            nc.tensor.matmul(
                qt_ps[:], xb[:, cb, bass.ts(k, P)], wct_sb[:, cb, :],
                start=(cb == 0), stop=(cb == CB - 1),
            )
        qt_sb = qt_pool.tile([P, 512], BF16, tag="qt")
        nc.scalar.copy(qt_sb[:], qt_ps[:])
        for ci in range(CB):
            nc.tensor.matmul(
                e_ps[ci][:], qt_sb[:, bass.ts(ci, P)], qt_sb[:],
                start=(k == 0), stop=(k == NK - 1),
            )

    # ---- Phase C: P = exp(E - m) with row-sum s; A' = P - diag(s) ----
    pb, srec = [], []
    for ci in range(CB):
        negmax = stat_pool.tile([P, 1], F32, tag="negmax")
        nc.vector.reduce_max(negmax[:], e_ps[ci][:], axis=AX, negate=True)
        pb_t = ab_pool.tile([P, 512], BF16, tag="ab")
        ssum = stat_pool.tile([P, 1], F32, tag="ssum")
        nc.scalar.activation(pb_t[:], e_ps[ci][:], EXP, bias=negmax[:],
                             scale=1.0, accum_out=ssum[:])
        sr = stat_pool.tile([P, 1], F32, tag="srec")
        nc.vector.reciprocal(sr[:], ssum[:])
        si = si_pool.tile([P, P], F32, tag="si")
        nc.vector.tensor_scalar_mul(si[:], ident_bf[:], ssum[:])
        nc.vector.tensor_sub(pb_t[:, bass.ts(ci, P)],
                             pb_t[:, bass.ts(ci, P)], si[:])
        pb.append(pb_t)
        srec.append(sr)

    # ---- Phase D: A'^T via PE transposes ----
    atb = []
    for dj in range(CB):
        at_ps = atps.tile([P, 512], BF16, tag="wps")
        for ci in range(CB):
            nc.tensor.transpose(at_ps[:, bass.ts(ci, P)],
                                pb[ci][:, bass.ts(dj, P)], ident_bf[:])
        at_sb = at_pool.tile([P, 512], BF16, tag="at")
        nc.vector.tensor_copy(out=at_sb[:], in_=at_ps[:])
        atb.append(at_sb)

    # ---- Phase E: out = (A'^T.T @ xb) * (1/s) + x, 8 n-chunks ----
    for j in range(NJ):
        for cb in range(CB):
            o_ps = ops.tile([P, 512], F32, tag="wps")
            for dj in range(CB):
                nc.tensor.matmul(
                    o_ps[:], atb[dj][:, bass.ts(cb, P)],
                    xb[:, dj, bass.ts(j, 512)],
                    start=(dj == 0), stop=(dj == CB - 1),
                )
            o_sb = out_pool.tile([P, 512], F32, tag="osb")
            nc.vector.scalar_tensor_tensor(
                out=o_sb[:], in0=o_ps[:], scalar=srec[cb][:],
                in1=xf2[j][:, cb, :], op0=MUL, op1=ADD)
            nc.sync.dma_start(ov[:, cb, bass.ts(j, 512)], o_sb[:])


def build_nc():
    nc = bacc.Bacc("TRN2", target_bir_lowering=False, debug=False)
    x_in = nc.dram_tensor("x_shard", [BPC, C, HW], F32,
                          kind="ExternalInput").ap()
    wct_in = nc.dram_tensor("wct", [C, C], BF16, kind="ExternalInput").ap()
    out_t = nc.dram_tensor("out", [BPC, C, HW], F32,
                           kind="ExternalOutput").ap()

    with tile.TileContext(nc) as tc:
        with ExitStack() as ctx:
            ec = ctx.enter_context
            const_pool = ec(tc.tile_pool(name="const", bufs=1))
            xb_pool = ec(tc.tile_pool(name="xb", bufs=2))
            qt_pool = ec(tc.tile_pool(name="qt", bufs=4))
            ab_pool = ec(tc.tile_pool(name="ab", bufs=8))
            at_pool = ec(tc.tile_pool(name="at", bufs=8))
            si_pool = ec(tc.tile_pool(name="si", bufs=2))
            stat_pool = ec(tc.tile_pool(name="stat", bufs=12))
            xf2_pool = ec(tc.tile_pool(name="xf2", bufs=8))
            out_pool = ec(tc.tile_pool(name="out", bufs=6))
            epsum = ec(tc.tile_pool(name="epsum", bufs=1, space="PSUM"))
            qtps = ec(tc.tile_pool(name="qtps", bufs=2, space="PSUM"))
            wps = ec(tc.tile_pool(name="wps", bufs=2, space="PSUM"))
            pools = (xb_pool, qt_pool, ab_pool, at_pool, si_pool,
                     stat_pool, xf2_pool, out_pool, qtps, epsum, wps, wps)

            ident_bf = const_pool.tile([P, P], BF16, tag="ident")
            make_identity(nc, ident_bf[:])
            wct_sb = const_pool.tile([P, CB, C], BF16, tag="wct")
            nc.sync.dma_start(
                wct_sb[:], wct_in.rearrange("(cb p) o -> p cb o", p=P))

            for b in range(BPC):
                xv = x_in[b].rearrange("(cb p) n -> p cb n", p=P)
                ov = out_t[b].rearrange("(cb p) n -> p cb n", p=P)
                _batch_body(ctx, tc, pools, xv, ov, wct_sb, ident_bf)
    nc.compile()
    return nc


_NC_CACHE = []


def _run(x: np.ndarray, Wc: np.ndarray, **spmd_kwargs):
    assert x.shape == (B, C, H, W) and x.dtype == np.float32
    if not _NC_CACHE:
        _NC_CACHE.append(build_nc())
    nc = _NC_CACHE[0]

    x_flat = np.ascontiguousarray(x.reshape(B, C, HW))
    wct = np.ascontiguousarray(Wc.T).astype(ml_dtypes.bfloat16)
    in_maps = [
        {"x_shard": x_flat[i * BPC:(i + 1) * BPC], "wct": wct}
        for i in range(N_CORES)
    ]
    res = run_bass_kernel_spmd(nc, in_maps, core_ids=list(range(N_CORES)),
                               **spmd_kwargs)
    out = np.concatenate([r["out"] for r in res.results], axis=0)
    return out.reshape(B, C, H, W), res


def kernel(x: np.ndarray, Wc: np.ndarray) -> np.ndarray:
    return _run(x, Wc)[0]


if __name__ == "__main__":
    nc = build_nc()
    print("built ok")
